# revision 40
# baseline (speedup 1.0000x reference)
# Bass/Tile Trainium2 kernel for batched multi-head causal self-attention.
#
# Problem: x[B=2,T=2048,C=1024], 16 heads (hd=64), causal softmax attention,
# output projection. Full (unsharded) inputs in, full output out.
#
# Sharding (Megatron-style): 8 cores = 2 batch groups x 4 head groups.
# Core i handles batch b = i // 4 and heads [4*(i%4) : 4*(i%4)+4).
# Each core computes Q/K/V projections for its 4 heads, causal attention,
# and a partial output projection (contribution of its heads).  The host
# sums the 4 partials per batch (the Megatron all-reduce) and adds bias.
#
# Schedule: a software pipeline interleaved at ~0.5us granularity.
# Attention is ACT(exp)-bound, so QKV matmuls for t-block tb+1 and the
# output-projection matmuls for q-block qb-1 are emitted as "filler"
# chunks between the S and P@V matmuls of q-block qb, keeping the PE
# busy while the Scalar engine chews through exp.
#
# On-device layout notes:
#   - Everything is kept "transposed" (feature dim on partitions):
#     xT [C, T], QT/KT [64, T] per head.  Heads come in pairs packed on
#     the 128 partitions (even head at [0:64], odd head at [64:128]); the
#     K=64 S^T matmuls of a pair use explicit tile_position row groups so
#     they run concurrently on disjoint PE quadrants.
#   - V is stored [T, 64] per head augmented with a ones column (V') so
#     the P@V matmul also produces the softmax denominator (row 64).
#   - Softmax runs without max-subtraction (scores are bounded ~|10|, exp
#     is safe in fp32), so no partition-dim reductions are ever needed.
#   - S^T tiles are per-k-tile [128, 1024] = (slot, q) PSUM tiles (2
#     banks), double-buffered so S(kt+1) overlaps exp(kt); exp is ONE
#     activation instruction per k-tile covering both slots.
#   - Causal masking: k-tiles strictly above the diagonal are skipped;
#     tiles crossing the diagonal get a triangular mask multiply and a
#     column-restricted P@V matmul.
#   - Startup: input DMAs are spread across the three DMA-capable queues
#     (SP, Activation, SWDGE) and the startup tensors are split into
#     SEPARATE tiles per chunk so the first matmuls are gated on their
#     own chunk's DMA, not the whole tensor (~10us earlier start).  A
#     burst of dummy matmuls right after sequencer init pre-warms the
#     PE HAM clock gate so the first real matmuls run at 2.4 GHz.
#   - Tail: the last q-block's projection phase-1 (hp0) matmuls are
#     emitted BEFORE the final normalize so they aren't serialized
#     behind it by engine-position semaphores; the final normalize runs
#     a DVE-only chain (row copies + reciprocal + cast) feeding a K=1
#     broadcast matmul instead of den-DMA hops + GpSimd broadcasts.
#   - QKV+attention internals are bf16; the projection also runs in bf16
#     (O^T normalized into bf16, Wp bf16) and partials are DMA'd out in
#     bf16; the host accumulates in fp32.

import numpy as np
from collections import deque

import concourse.bass as bass
import concourse.tile as tile
from concourse import bacc, mybir
from concourse import bass_utils

F32 = mybir.dt.float32
BF16 = mybir.dt.bfloat16
ATT_DT = BF16

B, T, C, H = 2, 2048, 1024, 16
HD = C // H            # 64 head dim
NCORES = 8
HPC = 4                # heads per core
DSEL = HPC * HD        # 256 feature dims per core
NTT = T // 128         # 16 t-tiles of 128
NCC = C // 128         # 8 c-chunks of 128
NQB = T // 512         # 4 q-blocks of 512


def build_program():
    nc = bacc.Bacc("TRN2", target_bir_lowering=False, debug=False)

    xT = nc.dram_tensor("xT", [128, NQB, NCC, 512], BF16, kind="ExternalInput").ap()
    wqT = nc.dram_tensor("wqT", [128, NCC * DSEL], BF16, kind="ExternalInput").ap()
    wkT = nc.dram_tensor("wkT", [128, NCC * DSEL], BF16, kind="ExternalInput").ap()
    wvT = nc.dram_tensor("wvT", [128, NCC * DSEL], BF16, kind="ExternalInput").ap()
    wpT = nc.dram_tensor("wpT", [128, 2 * C], BF16, kind="ExternalInput").ap()
    maskd = nc.dram_tensor("maskd", [128, 256], ATT_DT, kind="ExternalInput").ap()
    out_p = nc.dram_tensor("out_p", [T, C], BF16, kind="ExternalOutput").ap()

    scale = 1.0 / float(np.sqrt(HD))

    with tile.TileContext(nc) as tc:
        with (
            tc.tile_pool(name="consts", bufs=1) as consts,
            tc.tile_pool(name="persist", bufs=1) as persist,
            tc.tile_pool(name="pt", bufs=4) as ptpool,
            tc.tile_pool(name="psn", bufs=8) as psn,
            tc.tile_pool(name="nsm", bufs=6) as nsm,
            tc.tile_pool(name="outst", bufs=6) as outst,
            tc.tile_pool(name="sgp", bufs=2, space="PSUM") as sgp,
            tc.tile_pool(name="pop", bufs=1, space="PSUM") as pop,
            tc.tile_pool(name="gpp", bufs=2, space="PSUM") as gpp,
        ):
            # ---- startup tiles ----------------------------------------
            wq_sb = consts.tile([128, NCC, DSEL], BF16, tag="wq")
            wk_sb = consts.tile([128, NCC, DSEL], BF16, tag="wk")
            wv_sb = consts.tile([128, NCC, DSEL], BF16, tag="wv")
            wp_sb = consts.tile([128, 2, C], BF16, tag="wp")
            mk_sb = consts.tile([128, 256], ATT_DT, tag="mk")
            # x block 0 in two half-tiles so the first Q matmuls are gated
            # on a 512KB DMA, not the full 1MB
            xt0 = [persist.tile([128, 4, 512], BF16, tag=f"xt0{h}",
                                name=f"xt0{h}") for h in range(2)]
            xt_all = persist.tile([128, NQB - 1, NCC, 512], BF16, tag="xt")

            def xchunk(tb, cc):
                if tb == 0:
                    return xt0[cc // 4][:, cc % 4, :]
                return xt_all[:, tb - 1, cc, :]

            # ---- input DMAs: three queues, priority order -------------
            # Startup is DMA-bandwidth-bound (~200 GB/s effective with
            # all 8 cores pulling), so use whole-tensor transfers (big
            # descriptors) spread over the three DMA-capable queues.
            # scalar: x0 (trigger before the exp-table prewarm, so it
            #   isn't stuck behind the ~2.7us table load)
            # sync:   wq then wk;  gpsimd: mask, wv, wp
            x0s = xT[:, 0].rearrange("p c t -> p (c t)").rearrange(
                "p (h d) -> p h d", h=2)
            for h in range(2):
                nc.scalar.dma_start(
                    out=xt0[h][:].rearrange("p c t -> p (c t)"),
                    in_=x0s[:, h])
            nc.sync.dma_start(
                out=wq_sb[:].rearrange("p c d -> p (c d)"), in_=wqT[:])
            nc.gpsimd.dma_start(out=mk_sb[:], in_=maskd)
            nc.sync.dma_start(
                out=wk_sb[:].rearrange("p c d -> p (c d)"), in_=wkT[:])
            nc.gpsimd.dma_start(
                out=wv_sb[:].rearrange("p c d -> p (c d)"), in_=wvT[:])
            # wp is first needed ~70us in (attn(3)'s projection fillers):
            # behind wk on sync keeps it out of the startup-critical
            # window, where per-core HBM bandwidth is the limit
            nc.sync.dma_start(
                out=wp_sb[:].rearrange("p h c -> p (h c)"), in_=wpT[:])

            # ---- ACT exp table pre-warm (one-time ~2.7us table DMA) ----
            wrm_in = consts.tile([1, 16], F32, tag="wrm_in")
            wrm_out = consts.tile([1, 16], BF16, tag="wrm_out")
            nc.vector.memset(wrm_in[:], 0.0)
            nc.scalar.activation(
                out=wrm_out[:], in_=wrm_in[:],
                func=mybir.ActivationFunctionType.Exp, scale=1.0,
            )

            # ---- PE HAM clock-gate pre-warm ---------------------------
            # ~4us of dependency-free FULL-ARRAY matmuls right after
            # sequencer init so the PE is at 2.4 GHz when the first real
            # matmul issues (the HAM needs ~3.4us of sustained activity;
            # thin M=1 matmuls don't register enough activity).  The
            # startup DMAs pace the real work until ~14us anyway, so
            # these fill otherwise-idle time.
            warm_sb = consts.tile([128, 512], BF16, tag="warm")
            nc.vector.memset(warm_sb[:], 1.0)
            wtile = gpp.tile([128, 512], F32, tag="gp", name="wtile")
            for _ in range(10):
                nc.tensor.matmul(
                    wtile[:], warm_sb[:, 0:128], warm_sb[:],
                    start=True, stop=True, skip_group_check=True,
                )

            # persistent activations: QT/KT/OT head pairs packed on
            # partitions ([0:64] even slot, [64:128] odd slot), free = t
            qt_sb = persist.tile([128, 2, T], ATT_DT, tag="qt")
            kt_sb = persist.tile([128, 2, T], ATT_DT, tag="kt")
            ot_sb = persist.tile([128, 2, T], ATT_DT, tag="ot")
            # V' per k-tile: 4 heads x (64 V cols + 1 ones col)
            v_sb = persist.tile([128, NTT, HPC * (HD + 1)], ATT_DT, tag="v")

            ones_sb = consts.tile([128, NTT], F32, tag="ones")
            nc.vector.memset(ones_sb[:], 1.0)
            for h in range(HPC):
                nc.vector.tensor_copy(
                    out=v_sb[:, :, h * 65 + 64 : h * 65 + 65],
                    in_=ones_sb[:].rearrange("p (t o) -> p t o", o=1),
                )

            # ---- filler generators (PE work interleaved into attention)
            def gen_qk(tb):
                ts = slice(tb * 512, tb * 512 + 512)
                # Q then K: one 512-wide matmul chain per head-pair
                for wi, (wsb, dst) in enumerate(((wq_sb, qt_sb), (wk_sb, kt_sb))):
                    for pr in range(2):
                        acc = gpp.tile([128, 512], F32, tag="gp", name="acc")
                        for cc in range(NCC):
                            nc.tensor.matmul(
                                acc[:],
                                wsb[:, cc, pr * 128 : pr * 128 + 128],
                                xchunk(tb, cc),
                                start=(cc == 0), stop=(cc == NCC - 1),
                            )
                            if cc % 3 == 2:
                                yield
                        if mode["drain"] and (wi + pr) % 2 == 0:
                            # drain copies split across scalar and vector:
                            # all-scalar would delay the next block's exps,
                            # all-vector would back up DVE into a PSUM WAR
                            nc.scalar.copy(dst[:, pr, ts], acc[:])
                        else:
                            nc.vector.tensor_copy(out=dst[:, pr, ts], in_=acc[:])
                        yield

            def gen_v(tb):
                # V: [t, d] layout, two 256-t halves of the block
                for half in range(2):
                    accv = gpp.tile([128, 512], F32, tag="gp", name="accv")
                    for cc in range(NCC):
                        for tt in range(2):
                            tl = half * 256 + tt * 128
                            nc.tensor.matmul(
                                accv[:, tt * 256 : tt * 256 + 256],
                                xchunk(tb, cc)[:, tl : tl + 128],
                                wv_sb[:, cc, :],
                                start=(cc == 0 and tt == 0),
                                stop=(cc == NCC - 1 and tt == 1),
                            )
                        if cc % 3 == 2:
                            yield
                    t4 = tb * 4 + half * 2
                    pv4 = accv[:].rearrange("p (tt h d) -> p tt h d", tt=2, h=HPC)
                    vdst = v_sb[:, t4 : t4 + 2, :].rearrange(
                        "p tt (h e) -> p tt h e", h=HPC)
                    nc.vector.tensor_copy(
                        out=vdst[:, :, :, 0:HD], in_=pv4)
                    yield

            def gen_proj(qb):
                for tt in range(4 * qb, 4 * qb + 4):
                    tloc = slice(tt * 128, tt * 128 + 128)
                    pc0 = gpp.tile([128, 512], F32, tag="gp", name="pc0")
                    pc1 = gpp.tile([128, 512], F32, tag="gp", name="pc1")
                    for hpp in range(2):
                        for cb, pc in enumerate((pc0, pc1)):
                            nc.tensor.matmul(
                                pc[:],
                                ot_sb[:, hpp, tloc],
                                wp_sb[:, hpp, cb * 512 : cb * 512 + 512],
                                start=(hpp == 0), stop=(hpp == 1),
                            )
                        yield
                    ob = outst.tile([128, 1024], BF16, tag="ob", name="ob")
                    nc.vector.tensor_copy(out=ob[:, 0:512], in_=pc0[:])
                    nc.vector.tensor_copy(out=ob[:, 512:1024], in_=pc1[:])
                    # alternate queues so 4MB of output doesn't back up
                    # one queue into the tail
                    eng = nc.sync if tt % 2 == 0 else nc.gpsimd
                    eng.dma_start(
                        out=out_p[tt * 128 : tt * 128 + 128, :], in_=ob[:])
                    yield

            fillers = deque()
            warm = {"po": None}
            mode = {"drain": False}
            pending = {"fn": None}

            def pump(n):
                while n > 0 and fillers:
                    try:
                        next(fillers[0])
                        n -= 1
                    except StopIteration:
                        fillers.popleft()
                if n > 0 and warm["po"] is not None:
                    # PE idle keep-alive: dependency-free matmuls into po's
                    # unused partition rows (PV only writes rows 0:65) so the
                    # HAM clock gate never sees an idle window and re-throttles
                    for _ in range(min(n, 2)):
                        nc.tensor.matmul(
                            warm["po"][96:97, 0:64],
                            mk_sb[0:64, 0:1], mk_sb[0:64, 0:64],
                            start=True, stop=True, skip_group_check=True,
                            tile_position=(0, 96),
                        )

            def drain(gen):
                # ACT idles between blocks; give it the drained copies so
                # the Vector queue doesn't back up and stall PE on PSUM WAR
                mode["drain"] = True
                while gen in fillers:
                    pump(1)
                mode["drain"] = False

            # ---- attention per (q-block, head-pair) -------------------
            def emit_attn(qb, tail_hook=None):
                qs = slice(qb * 512, qb * 512 + 512)
                n_kt = 4 * (qb + 1)
                for hp in range(2):
                    po = pop.tile([128, 1024], F32, tag="po", name="po")
                    # keep-alives must not touch the new po slot until the
                    # previous head-pair's deferred normalize has read it
                    warm["po"] = None
                    prev_pv = None
                    for kt in range(n_kt):
                        pump(1)
                        if qb >= 2 and kt >= 2:
                            # PE keep-alive for the exp-paced late blocks:
                            # early blocks are PE-paced and don't need it
                            # (and at kt<2 the previous head-pair's po may
                            # still be draining into its normalize)
                            nc.tensor.matmul(
                                po[96:97, 0:64],
                                mk_sb[0:64, 0:1], mk_sb[0:64, 0:64],
                                start=True, stop=True, skip_group_check=True,
                                tile_position=(0, 96),
                            )
                        sg = sgp.tile([128, 1024], F32, tag="sg", name="sg")
                        pt = ptpool.tile([128, 1024], ATT_DT, tag="pt", name="pt")
                        j = kt - 4 * qb
                        roff = 128 * j if j >= 0 else 0
                        # S^T: 2 matmuls, row-split pair runs concurrently
                        for s in range(2):
                            psl = slice(64 * s, 64 * s + 64)
                            nc.tensor.matmul(
                                sg[:, s * 512 + roff : s * 512 + 512],
                                kt_sb[psl, hp, kt * 128 : kt * 128 + 128],
                                qt_sb[psl, hp, qb * 512 + roff : qb * 512 + 512],
                                start=True, stop=True,
                                tile_position=(64 * s, 0),
                            )
                        # exp: one instruction per k-tile; diagonal tiles
                        # restricted to the causal (written) columns
                        sgv = sg[:].rearrange("p (s q) -> p s q", s=2)
                        ptv = pt[:].rearrange("p (s q) -> p s q", s=2)
                        if j >= 0:
                            nc.scalar.activation(
                                out=ptv[:, :, 128 * j : 512],
                                in_=sgv[:, :, 128 * j : 512],
                                func=mybir.ActivationFunctionType.Exp,
                                scale=scale,
                            )
                            # triangular mask on the diagonal 128-col chunk
                            mk3 = mk_sb[:].rearrange("p (s q) -> p s q", s=2)
                            nc.vector.tensor_mul(
                                ptv[:, :, 128 * j : 128 * j + 128],
                                ptv[:, :, 128 * j : 128 * j + 128],
                                mk3,
                            )
                        else:
                            nc.scalar.activation(
                                out=pt[:], in_=sg[:],
                                func=mybir.ActivationFunctionType.Exp,
                                scale=scale,
                            )
                        if kt == 1:
                            if pending["fn"] is not None:
                                # previous head-pair's deferred normalize:
                                # emitting it one k-tile into THIS block
                                # lets the new block's S/exp start with no
                                # DVE wait
                                pending["fn"]()
                                pending["fn"] = None
                            warm["po"] = po
                        if qb < 2:
                            pump(1)
                        if prev_pv is not None:
                            prev_pv()

                        def pv(kt=kt, pt=pt, roff=roff):
                            for s in range(2):
                                h = 2 * hp + s
                                nc.tensor.matmul(
                                    po[0:65, s * 512 + roff : s * 512 + 512],
                                    v_sb[:, kt, h * 65 : h * 65 + 65],
                                    pt[:, s * 512 + roff : (s + 1) * 512],
                                    start=(kt == 0), stop=(kt == n_kt - 1),
                                )
                        prev_pv = pv
                    pump(1)
                    prev_pv()
                    if tail_hook is not None and hp == 1:
                        # final block: run the projection phase-1 matmuls
                        # (and leftover fillers) BEFORE the normalize ops
                        # so engine-position semaphores don't serialize
                        # them behind the normalize chain
                        tail_hook(po)
                        return
                    # O^T + denominator row out of PSUM, then normalize
                    # this head-pair.  The den row is moved to partition
                    # 0 with a DVE shifted copy (engines may use distinct
                    # partition bases per operand; gpsimd broadcast can
                    # only read partition 0, and DMA hops cost ~1.3us).
                    # Deferred into the next block so its S/exp pipeline
                    # restarts with no DVE wait.
                    def norm(po=po, hp=hp, qs=qs):
                        psq = {}
                        for s in range(2):
                            ps_sb = psn.tile([65, 512], F32, tag="ps",
                                             name=f"ps{hp}{s}")
                            nc.vector.tensor_copy(
                                out=ps_sb[:],
                                in_=po[0:65, s * 512 : s * 512 + 512])
                            psq[s] = ps_sb
                        for s in range(2):
                            dn = nsm.tile([1, 512], F32, tag="dn", name="dn")
                            rf = nsm.tile([1, 512], F32, tag="rf", name="rf")
                            rb = nsm.tile([64, 512], F32, tag="rb", name="rb")
                            nc.vector.tensor_copy(out=dn[:], in_=psq[s][64:65, :])
                            nc.vector.reciprocal_approx_fast(rf[:], dn[:])
                            nc.gpsimd.partition_broadcast(rb[:], rf[:])
                            nc.vector.tensor_mul(
                                ot_sb[64 * s : 64 * s + 64, hp, qs],
                                psq[s][0:64, :],
                                rb[:],
                            )
                    pending["fn"] = norm

            # ---- tail: projection of the last q-block -----------------
            def tail(po):
                # Phase 1 (emitted before the final normalize): leftover
                # fillers, then hp0-phase matmuls for three of the four
                # projection tiles (tt12/13 in the two sg slots, tt14 in
                # the po slot once the ps copies release it).
                warm["po"] = None
                mode["drain"] = True
                while fillers:
                    pump(1)
                mode["drain"] = False
                tts = list(range(4 * (NQB - 1), 4 * NQB))
                # normalize chain for (qb3, hp1): po readers first (the
                # V+den copies, split across scalar/vector) so the po
                # slot frees early for tt14's phase-1; reciprocals run
                # on the den rows in place (partition 64), broadcasts
                # read them directly (no DMA hops).
                qb = NQB - 1
                psq = {}
                rfs = {}
                for s in range(2):
                    ps_sb = psn.tile([65, 512], F32, tag="ps", name=f"pst{s}")
                    if s == 0:
                        nc.scalar.copy(
                            ps_sb[:], po[0:65, s * 512 : s * 512 + 512])
                    else:
                        nc.vector.tensor_copy(
                            out=ps_sb[:],
                            in_=po[0:65, s * 512 : s * 512 + 512])
                    psq[s] = ps_sb
                for s in range(2):
                    dn = nsm.tile([1, 512], F32, tag="dn", name=f"dnt{s}")
                    rf = nsm.tile([1, 512], F32, tag="rf", name=f"rft{s}")
                    nc.vector.tensor_copy(out=dn[:], in_=psq[s][64:65, :])
                    nc.vector.reciprocal_approx_fast(rf[:], dn[:])
                    rfs[s] = rf
                accs = {}
                for i, tt in enumerate(tts[:3]):
                    tloc = slice(tt * 128, tt * 128 + 128)
                    if i < 2:
                        big = sgp.tile([128, 1024], F32, tag="sg", name="prj")
                    else:
                        big = pop.tile([128, 1024], F32, tag="po", name="prjp")
                    accs[tt] = (big[:, 0:512], big[:, 512:1024])
                    for cb, pc in enumerate(accs[tt]):
                        nc.tensor.matmul(
                            pc[:],
                            ot_sb[:, 0, tloc],
                            wp_sb[:, 0, cb * 512 : cb * 512 + 512],
                            start=True, stop=False,
                        )
                # PE keep-alives through the normalize window so the
                # phase-2 matmuls issue at full clock (all real PSUM
                # banks hold open accumulation groups; use a gp slot)
                katile = gpp.tile([128, 512], F32, tag="gp", name="katile")
                for _ in range(20):
                    nc.tensor.matmul(
                        katile[96:97, 0:64],
                        mk_sb[0:64, 0:1], mk_sb[0:64, 0:64],
                        start=True, stop=True, skip_group_check=True,
                        tile_position=(0, 96),
                    )
                # broadcast rec (gpsimd), then normalize per 128-col
                # chunk (both slots), so each projection tile can go as
                # soon as its columns are done
                rbs = {}
                for s in range(2):
                    rb = nsm.tile([64, 512], F32, tag="rb", name=f"rbt{s}")
                    nc.gpsimd.partition_broadcast(rb[:], rfs[s][:])
                    rbs[s] = rb
                for ch in range(4):
                    for s in range(2):
                        cs = slice(qb * 512 + ch * 128, qb * 512 + ch * 128 + 128)
                        nc.vector.tensor_mul(
                            ot_sb[64 * s : 64 * s + 64, 1, cs],
                            psq[s][0:64, ch * 128 : ch * 128 + 128],
                            rbs[s][:, ch * 128 : ch * 128 + 128],
                        )
                # Phase 2: hp1-phase matmuls, then per-tile casts split
                # across scalar+vector and per-HALF output DMAs on
                # rotating queues, so the final drain is latency- not
                # serialization-bound.
                dmaq = [nc.sync, nc.scalar, nc.gpsimd]
                qi = 0
                for i, tt in enumerate(tts[:3]):
                    tloc = slice(tt * 128, tt * 128 + 128)
                    for cb, pc in enumerate(accs[tt]):
                        nc.tensor.matmul(
                            pc[:],
                            ot_sb[:, 1, tloc],
                            wp_sb[:, 1, cb * 512 : cb * 512 + 512],
                            start=False, stop=True,
                        )
                    ob = outst.tile([128, 1024], BF16, tag="ob", name="obt")
                    nc.scalar.copy(ob[:, 0:512], accs[tt][0][:])
                    nc.vector.tensor_copy(out=ob[:, 512:1024], in_=accs[tt][1][:])
                    for cb in range(2):
                        dmaq[qi % 3].dma_start(
                            out=out_p[tt * 128 : tt * 128 + 128,
                                      cb * 512 : cb * 512 + 512],
                            in_=ob[:, cb * 512 : cb * 512 + 512])
                        qi += 1
                # tt15: both phases back-to-back in the remaining gp slot
                tt = tts[3]
                tloc = slice(tt * 128, tt * 128 + 128)
                ob = outst.tile([128, 1024], BF16, tag="ob", name="obt3")
                for cb in range(2):
                    pc = gpp.tile([128, 512], F32, tag="gp", name=f"pt3{cb}")
                    for hpp in range(2):
                        nc.tensor.matmul(
                            pc[:],
                            ot_sb[:, hpp, tloc],
                            wp_sb[:, hpp, cb * 512 : cb * 512 + 512],
                            start=(hpp == 0), stop=(hpp == 1),
                        )
                    if cb == 0:
                        nc.scalar.copy(ob[:, 0:512], pc[:])
                    else:
                        nc.vector.tensor_copy(
                            out=ob[:, 512:1024], in_=pc[:])
                    dmaq[qi % 3].dma_start(
                        out=out_p[tt * 128 : tt * 128 + 128,
                                  cb * 512 : cb * 512 + 512],
                        in_=ob[:, cb * 512 : cb * 512 + 512])
                    qi += 1

            # ---- main pipeline ----------------------------------------
            for _ in gen_qk(0):
                pass
            gq = {}
            fillers.append(gen_v(0))
            for qb in range(NQB):
                if qb < NQB - 1:
                    # stage the next t-block of x now: it rides the Scalar
                    # queue ahead of this block's exps and transfers while
                    # attn(qb) runs, without competing with wq/x0 at start
                    nc.scalar.dma_start(
                        out=xt_all[:, qb].rearrange("p c t -> p (c t)"),
                        in_=xT[:, qb + 1].rearrange("p c t -> p (c t)"))
                    gq[qb + 1] = gen_qk(qb + 1)
                    fillers.append(gq[qb + 1])
                    fillers.append(gen_v(qb + 1))
                else:
                    # attn(3) is the longest ACT-bound stretch and has no
                    # QKV left to interleave: feed it all three finished
                    # projection blocks to keep the PE warm.  attn(2)'s
                    # deferred normalize must be flushed first: gen_proj(2)
                    # reads the O^T columns it writes, and a filler chunk
                    # emitted before it would read pre-normalize data.
                    if pending["fn"] is not None:
                        pending["fn"]()
                        pending["fn"] = None
                    for pq_ in range(NQB - 1):
                        fillers.append(gen_proj(pq_))
                emit_attn(qb, tail_hook=tail if qb == NQB - 1 else None)
                # attn(qb+1) S-matmuls read qt/kt of t-block qb+1, so those
                # must be fully emitted first; V is only read by late P@V
                # groups and can keep riding as filler.
                if qb < NQB - 1:
                    drain(gq[qb + 1])

    nc.compile()
    return nc


_NC_CACHE = None


def _get_program():
    global _NC_CACHE
    if _NC_CACHE is None:
        _NC_CACHE = build_program()
    return _NC_CACHE


def make_in_maps(x, Wq, Wk, Wv, Wp):
    import ml_dtypes
    x = np.asarray(x, np.float32)
    Wq = np.asarray(Wq, np.float32)
    Wk = np.asarray(Wk, np.float32)
    Wv = np.asarray(Wv, np.float32)
    Wp = np.asarray(Wp, np.float32)
    tri = np.triu(np.ones((128, 128), np.float32))  # mask[k,q] = (k<=q)
    maskd = np.concatenate([tri, tri], axis=1).astype(ml_dtypes.bfloat16)
    in_maps = []
    for core in range(NCORES):
        b, hg = core // 4, core % 4
        sel = slice(hg * DSEL, (hg + 1) * DSEL)
        # SBUF images: [128, cc, ...] with partition index innermost in
        # the original feature dim (feature c -> (cc, p))
        # [128, cc, T] -> [128, tb, cc, 512] (contiguous per t-block)
        xi = x[b].T.reshape(NCC, 128, T).transpose(1, 0, 2)
        xi = xi.reshape(128, NCC, NQB, 512).transpose(0, 2, 1, 3)
        wqi = Wq[sel, :].T.reshape(NCC, 128, DSEL).transpose(1, 0, 2).reshape(128, NCC * DSEL)
        wki = Wk[sel, :].T.reshape(NCC, 128, DSEL).transpose(1, 0, 2).reshape(128, NCC * DSEL)
        wvi = Wv[sel, :].T.reshape(NCC, 128, DSEL).transpose(1, 0, 2).reshape(128, NCC * DSEL)
        wpi = Wp[:, sel].T.reshape(2, 128, C).transpose(1, 0, 2).reshape(128, 2 * C)
        in_maps.append({
            "xT": np.ascontiguousarray(xi.astype(ml_dtypes.bfloat16)),
            "wqT": np.ascontiguousarray(wqi.astype(ml_dtypes.bfloat16)),
            "wkT": np.ascontiguousarray(wki.astype(ml_dtypes.bfloat16)),
            "wvT": np.ascontiguousarray(wvi.astype(ml_dtypes.bfloat16)),
            "wpT": np.ascontiguousarray(wpi.astype(ml_dtypes.bfloat16)),
            "maskd": maskd,
        })
    return in_maps


def combine_outputs(results, bp):
    parts = [np.asarray(results[i]["out_p"], np.float32) for i in range(NCORES)]
    out = np.stack([
        parts[0] + parts[1] + parts[2] + parts[3],
        parts[4] + parts[5] + parts[6] + parts[7],
    ])
    return (out + np.asarray(bp, np.float32)).astype(np.float32)


def kernel(x, Wq, Wk, Wv, Wp, bp):
    nc = _get_program()
    in_maps = make_in_maps(x, Wq, Wk, Wv, Wp)
    res = bass_utils.run_bass_kernel_spmd(nc, in_maps, core_ids=list(range(NCORES)))
    return combine_outputs(res.results, bp)


# revision 46
# speedup vs baseline: 1.0223x; 1.0223x over previous
# Bass/Tile Trainium2 kernel for batched multi-head causal self-attention.
#
# Problem: x[B=2,T=2048,C=1024], 16 heads (hd=64), causal softmax attention,
# output projection. Full (unsharded) inputs in, full output out.
#
# Sharding (Megatron-style): 8 cores = 2 batch groups x 4 head groups.
# Core i handles batch b = i // 4 and heads [4*(i%4) : 4*(i%4)+4).
# Each core computes Q/K/V projections for its 4 heads, causal attention,
# and a partial output projection (contribution of its heads).  The host
# sums the 4 partials per batch (the Megatron all-reduce) and adds bias.
#
# Schedule: a software pipeline interleaved at ~0.5us granularity.
# Attention is ACT(exp)-bound, so QKV matmuls for t-block tb+1 and the
# output-projection matmuls for q-block qb-1 are emitted as "filler"
# chunks between the S and P@V matmuls of q-block qb, keeping the PE
# busy while the Scalar engine chews through exp.
#
# On-device layout notes:
#   - Everything is kept "transposed" (feature dim on partitions):
#     xT [C, T], QT/KT [64, T] per head.  Heads come in pairs packed on
#     the 128 partitions (even head at [0:64], odd head at [64:128]); the
#     K=64 S^T matmuls of a pair use explicit tile_position row groups so
#     they run concurrently on disjoint PE quadrants.
#   - V is stored [T, 64] per head augmented with a ones column (V') so
#     the P@V matmul also produces the softmax denominator (row 64).
#   - Softmax runs without max-subtraction (scores are bounded ~|10|, exp
#     is safe in fp32), so no partition-dim reductions are ever needed.
#   - S^T tiles are per-k-tile [128, 1024] = (slot, q) PSUM tiles (2
#     banks), double-buffered so S(kt+1) overlaps exp(kt); exp is ONE
#     activation instruction per k-tile covering both slots.
#   - Causal masking: k-tiles strictly above the diagonal are skipped;
#     tiles crossing the diagonal get a triangular mask multiply and a
#     column-restricted P@V matmul.
#   - Startup: input DMAs are spread across the three DMA-capable queues
#     (SP, Activation, SWDGE) and the startup tensors are split into
#     SEPARATE tiles per chunk so the first matmuls are gated on their
#     own chunk's DMA, not the whole tensor (~10us earlier start).  A
#     burst of dummy matmuls right after sequencer init pre-warms the
#     PE HAM clock gate so the first real matmuls run at 2.4 GHz.
#   - Tail: the last q-block's projection phase-1 (hp0) matmuls are
#     emitted BEFORE the final normalize so they aren't serialized
#     behind it by engine-position semaphores; the final normalize runs
#     a DVE-only chain (row copies + reciprocal + cast) feeding a K=1
#     broadcast matmul instead of den-DMA hops + GpSimd broadcasts.
#   - QKV+attention internals are bf16; the projection also runs in bf16
#     (O^T normalized into bf16, Wp bf16) and partials are DMA'd out in
#     bf16; the host accumulates in fp32.

import numpy as np
from collections import deque

import concourse.bass as bass
import concourse.tile as tile
from concourse import bacc, mybir
from concourse import bass_utils

F32 = mybir.dt.float32
BF16 = mybir.dt.bfloat16
ATT_DT = BF16

B, T, C, H = 2, 2048, 1024, 16
HD = C // H            # 64 head dim
NCORES = 8
HPC = 4                # heads per core
DSEL = HPC * HD        # 256 feature dims per core
NTT = T // 128         # 16 t-tiles of 128
NCC = C // 128         # 8 c-chunks of 128
NQB = T // 512         # 4 q-blocks of 512


def build_program():
    nc = bacc.Bacc("TRN2", target_bir_lowering=False, debug=False)

    xT = nc.dram_tensor("xT", [128, NQB, NCC, 512], BF16, kind="ExternalInput").ap()
    wqT = nc.dram_tensor("wqT", [128, NCC * DSEL], BF16, kind="ExternalInput").ap()
    wkT = nc.dram_tensor("wkT", [128, NCC * DSEL], BF16, kind="ExternalInput").ap()
    wvT = nc.dram_tensor("wvT", [128, NCC * DSEL], BF16, kind="ExternalInput").ap()
    wpT = nc.dram_tensor("wpT", [128, 2 * C], BF16, kind="ExternalInput").ap()
    maskd = nc.dram_tensor("maskd", [128, 256], ATT_DT, kind="ExternalInput").ap()
    out_p = nc.dram_tensor("out_p", [T, C], BF16, kind="ExternalOutput").ap()

    scale = 1.0 / float(np.sqrt(HD))

    with tile.TileContext(nc) as tc:
        with (
            tc.tile_pool(name="consts", bufs=1) as consts,
            tc.tile_pool(name="persist", bufs=1) as persist,
            tc.tile_pool(name="pt", bufs=4) as ptpool,
            tc.tile_pool(name="psn", bufs=8) as psn,
            tc.tile_pool(name="nsm", bufs=6) as nsm,
            tc.tile_pool(name="outst", bufs=6) as outst,
            tc.tile_pool(name="sgp", bufs=2, space="PSUM") as sgp,
            tc.tile_pool(name="pop", bufs=1, space="PSUM") as pop,
            tc.tile_pool(name="gpp", bufs=2, space="PSUM") as gpp,
        ):
            # ---- startup tiles ----------------------------------------
            wq_sb = consts.tile([128, NCC, DSEL], BF16, tag="wq")
            wk_sb = consts.tile([128, NCC, DSEL], BF16, tag="wk")
            wv_sb = consts.tile([128, NCC, DSEL], BF16, tag="wv")
            wp_sb = consts.tile([128, 2, C], BF16, tag="wp")
            mk_sb = consts.tile([128, 256], ATT_DT, tag="mk")
            xt_all = persist.tile([128, NQB, NCC, 512], BF16, tag="xt")

            def xchunk(tb, cc):
                return xt_all[:, tb, cc, :]

            # ---- input DMAs: three queues, priority order -------------
            # Startup is DMA-bandwidth-bound (~200 GB/s effective with
            # all 8 cores pulling), so use whole-tensor transfers (big
            # descriptors) spread over the three DMA-capable queues.
            # scalar: x0 (trigger before the exp-table prewarm, so it
            #   isn't stuck behind the ~2.7us table load)
            # sync:   wq then wk;  gpsimd: mask, wv, wp
            nc.scalar.dma_start(
                out=xt_all[:, 0].rearrange("p c t -> p (c t)"),
                in_=xT[:, 0].rearrange("p c t -> p (c t)"))
            nc.sync.dma_start(
                out=wq_sb[:].rearrange("p c d -> p (c d)"), in_=wqT[:])
            nc.gpsimd.dma_start(out=mk_sb[:], in_=maskd)
            nc.sync.dma_start(
                out=wk_sb[:].rearrange("p c d -> p (c d)"), in_=wkT[:])
            nc.gpsimd.dma_start(
                out=wv_sb[:].rearrange("p c d -> p (c d)"), in_=wvT[:])
            nc.gpsimd.dma_start(
                out=wp_sb[:].rearrange("p h c -> p (h c)"), in_=wpT[:])

            # ---- ACT exp table pre-warm (one-time ~2.7us table DMA) ----
            wrm_in = consts.tile([1, 16], F32, tag="wrm_in")
            wrm_out = consts.tile([1, 16], BF16, tag="wrm_out")
            nc.vector.memset(wrm_in[:], 0.0)
            nc.scalar.activation(
                out=wrm_out[:], in_=wrm_in[:],
                func=mybir.ActivationFunctionType.Exp, scale=1.0,
            )

            # ---- PE HAM clock-gate pre-warm ---------------------------
            # ~4us of dependency-free FULL-ARRAY matmuls right after
            # sequencer init so the PE is at 2.4 GHz when the first real
            # matmul issues (the HAM needs ~3.4us of sustained activity;
            # thin M=1 matmuls don't register enough activity).  The
            # startup DMAs pace the real work until ~14us anyway, so
            # these fill otherwise-idle time.
            warm_sb = consts.tile([128, 512], BF16, tag="warm")
            nc.vector.memset(warm_sb[:], 1.0)
            wtile = gpp.tile([128, 512], F32, tag="gp", name="wtile")
            for _ in range(10):
                nc.tensor.matmul(
                    wtile[:], warm_sb[:, 0:128], warm_sb[:],
                    start=True, stop=True, skip_group_check=True,
                )

            # persistent activations: QT/KT/OT head pairs packed on
            # partitions ([0:64] even slot, [64:128] odd slot), free = t
            qt_sb = persist.tile([128, 2, T], ATT_DT, tag="qt")
            kt_sb = persist.tile([128, 2, T], ATT_DT, tag="kt")
            ot_sb = persist.tile([128, 2, T], ATT_DT, tag="ot")
            # V' per k-tile: 4 heads x (64 V cols + 1 ones col)
            v_sb = persist.tile([128, NTT, HPC * (HD + 1)], ATT_DT, tag="v")

            ones_sb = consts.tile([128, NTT], F32, tag="ones")
            nc.vector.memset(ones_sb[:], 1.0)
            for h in range(HPC):
                nc.vector.tensor_copy(
                    out=v_sb[:, :, h * 65 + 64 : h * 65 + 65],
                    in_=ones_sb[:].rearrange("p (t o) -> p t o", o=1),
                )

            # ---- filler generators (PE work interleaved into attention)
            def gen_qk(tb):
                ts = slice(tb * 512, tb * 512 + 512)
                # Q then K: one 512-wide matmul chain per head-pair
                for wsb, dst in ((wq_sb, qt_sb), (wk_sb, kt_sb)):
                    for pr in range(2):
                        acc = gpp.tile([128, 512], F32, tag="gp", name="acc")
                        for cc in range(NCC):
                            nc.tensor.matmul(
                                acc[:],
                                wsb[:, cc, pr * 128 : pr * 128 + 128],
                                xchunk(tb, cc),
                                start=(cc == 0), stop=(cc == NCC - 1),
                            )
                            if cc % 3 == 2:
                                yield
                        if mode["drain"]:
                            nc.scalar.copy(dst[:, pr, ts], acc[:])
                        else:
                            nc.vector.tensor_copy(out=dst[:, pr, ts], in_=acc[:])
                        yield

            def gen_v(tb):
                # V: [t, d] layout, two 256-t halves of the block
                for half in range(2):
                    accv = gpp.tile([128, 512], F32, tag="gp", name="accv")
                    for cc in range(NCC):
                        for tt in range(2):
                            tl = half * 256 + tt * 128
                            nc.tensor.matmul(
                                accv[:, tt * 256 : tt * 256 + 256],
                                xchunk(tb, cc)[:, tl : tl + 128],
                                wv_sb[:, cc, :],
                                start=(cc == 0 and tt == 0),
                                stop=(cc == NCC - 1 and tt == 1),
                            )
                        if cc % 3 == 2:
                            yield
                    t4 = tb * 4 + half * 2
                    pv4 = accv[:].rearrange("p (tt h d) -> p tt h d", tt=2, h=HPC)
                    vdst = v_sb[:, t4 : t4 + 2, :].rearrange(
                        "p tt (h e) -> p tt h e", h=HPC)
                    nc.vector.tensor_copy(
                        out=vdst[:, :, :, 0:HD], in_=pv4)
                    yield

            def gen_proj(qb):
                for tt in range(4 * qb, 4 * qb + 4):
                    tloc = slice(tt * 128, tt * 128 + 128)
                    pc0 = gpp.tile([128, 512], F32, tag="gp", name="pc0")
                    pc1 = gpp.tile([128, 512], F32, tag="gp", name="pc1")
                    for hpp in range(2):
                        for cb, pc in enumerate((pc0, pc1)):
                            nc.tensor.matmul(
                                pc[:],
                                ot_sb[:, hpp, tloc],
                                wp_sb[:, hpp, cb * 512 : cb * 512 + 512],
                                start=(hpp == 0), stop=(hpp == 1),
                            )
                        yield
                    ob = outst.tile([128, 1024], BF16, tag="ob", name="ob")
                    nc.vector.tensor_copy(out=ob[:, 0:512], in_=pc0[:])
                    nc.vector.tensor_copy(out=ob[:, 512:1024], in_=pc1[:])
                    # alternate queues so 4MB of output doesn't back up
                    # one queue into the tail
                    eng = nc.sync if tt % 2 == 0 else nc.gpsimd
                    eng.dma_start(
                        out=out_p[tt * 128 : tt * 128 + 128, :], in_=ob[:])
                    yield

            fillers = deque()
            warm = {"po": None}
            mode = {"drain": False}
            pending = {"fn": None}

            def pump(n):
                while n > 0 and fillers:
                    try:
                        next(fillers[0])
                        n -= 1
                    except StopIteration:
                        fillers.popleft()
                if n > 0 and warm["po"] is not None:
                    # PE idle keep-alive: dependency-free matmuls into po's
                    # unused partition rows (PV only writes rows 0:65) so the
                    # HAM clock gate never sees an idle window and re-throttles
                    for _ in range(min(n, 2)):
                        nc.tensor.matmul(
                            warm["po"][96:97, 0:64],
                            mk_sb[0:64, 0:1], mk_sb[0:64, 0:64],
                            start=True, stop=True, skip_group_check=True,
                            tile_position=(0, 96),
                        )

            def drain(gen):
                # ACT idles between blocks; give it the drained copies so
                # the Vector queue doesn't back up and stall PE on PSUM WAR
                mode["drain"] = True
                while gen in fillers:
                    pump(1)
                mode["drain"] = False

            # ---- attention per (q-block, head-pair) -------------------
            def emit_attn(qb, tail_hook=None):
                qs = slice(qb * 512, qb * 512 + 512)
                n_kt = 4 * (qb + 1)
                for hp in range(2):
                    po = pop.tile([128, 1024], F32, tag="po", name="po")
                    # keep-alives must not touch the new po slot until the
                    # previous head-pair's deferred normalize has read it
                    warm["po"] = None
                    prev_pv = None
                    for kt in range(n_kt):
                        pump(1)
                        if qb >= 2 and kt >= 2:
                            # PE keep-alive for the exp-paced late blocks:
                            # early blocks are PE-paced and don't need it
                            # (and at kt<2 the previous head-pair's po may
                            # still be draining into its normalize)
                            nc.tensor.matmul(
                                po[96:97, 0:64],
                                mk_sb[0:64, 0:1], mk_sb[0:64, 0:64],
                                start=True, stop=True, skip_group_check=True,
                                tile_position=(0, 96),
                            )
                        sg = sgp.tile([128, 1024], F32, tag="sg", name="sg")
                        pt = ptpool.tile([128, 1024], ATT_DT, tag="pt", name="pt")
                        j = kt - 4 * qb
                        roff = 128 * j if j >= 0 else 0
                        # S^T: 2 matmuls, row-split pair runs concurrently
                        for s in range(2):
                            psl = slice(64 * s, 64 * s + 64)
                            nc.tensor.matmul(
                                sg[:, s * 512 + roff : s * 512 + 512],
                                kt_sb[psl, hp, kt * 128 : kt * 128 + 128],
                                qt_sb[psl, hp, qb * 512 + roff : qb * 512 + 512],
                                start=True, stop=True,
                                tile_position=(64 * s, 0),
                            )
                        # exp: one instruction per k-tile; diagonal tiles
                        # restricted to the causal (written) columns
                        sgv = sg[:].rearrange("p (s q) -> p s q", s=2)
                        ptv = pt[:].rearrange("p (s q) -> p s q", s=2)
                        if j >= 0:
                            nc.scalar.activation(
                                out=ptv[:, :, 128 * j : 512],
                                in_=sgv[:, :, 128 * j : 512],
                                func=mybir.ActivationFunctionType.Exp,
                                scale=scale,
                            )
                            # triangular mask on the diagonal 128-col chunk
                            mk3 = mk_sb[:].rearrange("p (s q) -> p s q", s=2)
                            nc.vector.tensor_mul(
                                ptv[:, :, 128 * j : 128 * j + 128],
                                ptv[:, :, 128 * j : 128 * j + 128],
                                mk3,
                            )
                        else:
                            nc.scalar.activation(
                                out=pt[:], in_=sg[:],
                                func=mybir.ActivationFunctionType.Exp,
                                scale=scale,
                            )
                        if kt == 1:
                            if pending["fn"] is not None:
                                # previous head-pair's deferred normalize:
                                # emitting it one k-tile into THIS block
                                # lets the new block's S/exp start with no
                                # DVE wait
                                pending["fn"]()
                                pending["fn"] = None
                            warm["po"] = po
                        if qb < 2:
                            pump(1)
                        if prev_pv is not None:
                            prev_pv()

                        def pv(kt=kt, pt=pt, roff=roff):
                            for s in range(2):
                                h = 2 * hp + s
                                nc.tensor.matmul(
                                    po[0:65, s * 512 + roff : s * 512 + 512],
                                    v_sb[:, kt, h * 65 : h * 65 + 65],
                                    pt[:, s * 512 + roff : (s + 1) * 512],
                                    start=(kt == 0), stop=(kt == n_kt - 1),
                                )
                        prev_pv = pv
                    pump(1)
                    prev_pv()
                    if tail_hook is not None and hp == 1:
                        # final block: run the projection phase-1 matmuls
                        # (and leftover fillers) BEFORE the normalize ops
                        # so engine-position semaphores don't serialize
                        # them behind the normalize chain
                        tail_hook(po)
                        return
                    # O^T + denominator row out of PSUM, then normalize
                    # this head-pair.  The den row is moved to partition
                    # 0 with a DVE shifted copy (engines may use distinct
                    # partition bases per operand; gpsimd broadcast can
                    # only read partition 0, and DMA hops cost ~1.3us).
                    # Deferred into the next block so its S/exp pipeline
                    # restarts with no DVE wait.
                    def norm(po=po, hp=hp, qs=qs):
                        psq = {}
                        for s in range(2):
                            ps_sb = psn.tile([65, 512], F32, tag="ps",
                                             name=f"ps{hp}{s}")
                            nc.vector.tensor_copy(
                                out=ps_sb[:],
                                in_=po[0:65, s * 512 : s * 512 + 512])
                            psq[s] = ps_sb
                        for s in range(2):
                            dn = nsm.tile([1, 512], F32, tag="dn", name="dn")
                            rf = nsm.tile([1, 512], F32, tag="rf", name="rf")
                            rb = nsm.tile([64, 512], F32, tag="rb", name="rb")
                            nc.vector.tensor_copy(out=dn[:], in_=psq[s][64:65, :])
                            nc.vector.reciprocal_approx_fast(rf[:], dn[:])
                            nc.gpsimd.partition_broadcast(rb[:], rf[:])
                            nc.vector.tensor_mul(
                                ot_sb[64 * s : 64 * s + 64, hp, qs],
                                psq[s][0:64, :],
                                rb[:],
                            )
                    pending["fn"] = norm

            # ---- tail: projection of the last q-block -----------------
            def tail(po):
                # Phase 1 (emitted before the final normalize): leftover
                # fillers, then hp0-phase matmuls for three of the four
                # projection tiles (tt12/13 in the two sg slots, tt14 in
                # the po slot once the ps copies release it).
                warm["po"] = None
                mode["drain"] = True
                while fillers:
                    pump(1)
                mode["drain"] = False
                tts = list(range(4 * (NQB - 1), 4 * NQB))
                # normalize chain for (qb3, hp1): po readers first (the
                # V+den copies, split across scalar/vector) so the po
                # slot frees early for tt14's phase-1; reciprocals run
                # on the den rows in place (partition 64), broadcasts
                # read them directly (no DMA hops).
                qb = NQB - 1
                psq = {}
                rfs = {}
                for s in range(2):
                    ps_sb = psn.tile([65, 512], F32, tag="ps", name=f"pst{s}")
                    if s == 0:
                        nc.scalar.copy(
                            ps_sb[:], po[0:65, s * 512 : s * 512 + 512])
                    else:
                        nc.vector.tensor_copy(
                            out=ps_sb[:],
                            in_=po[0:65, s * 512 : s * 512 + 512])
                    psq[s] = ps_sb
                for s in range(2):
                    dn = nsm.tile([1, 512], F32, tag="dn", name=f"dnt{s}")
                    rf = nsm.tile([1, 512], F32, tag="rf", name=f"rft{s}")
                    nc.vector.tensor_copy(out=dn[:], in_=psq[s][64:65, :])
                    nc.vector.reciprocal_approx_fast(rf[:], dn[:])
                    rfs[s] = rf
                accs = {}
                for i, tt in enumerate(tts[:3]):
                    tloc = slice(tt * 128, tt * 128 + 128)
                    if i < 2:
                        big = sgp.tile([128, 1024], F32, tag="sg", name="prj")
                    else:
                        big = pop.tile([128, 1024], F32, tag="po", name="prjp")
                    accs[tt] = (big[:, 0:512], big[:, 512:1024])
                    for cb, pc in enumerate(accs[tt]):
                        nc.tensor.matmul(
                            pc[:],
                            ot_sb[:, 0, tloc],
                            wp_sb[:, 0, cb * 512 : cb * 512 + 512],
                            start=True, stop=False,
                        )
                # PE keep-alives through the normalize window so the
                # phase-2 matmuls issue at full clock (all real PSUM
                # banks hold open accumulation groups; use a gp slot)
                katile = gpp.tile([128, 512], F32, tag="gp", name="katile")
                for _ in range(20):
                    nc.tensor.matmul(
                        katile[96:97, 0:64],
                        mk_sb[0:64, 0:1], mk_sb[0:64, 0:64],
                        start=True, stop=True, skip_group_check=True,
                        tile_position=(0, 96),
                    )
                # broadcast rec (gpsimd), then normalize per 128-col
                # chunk (both slots), so each projection tile can go as
                # soon as its columns are done
                rbs = {}
                for s in range(2):
                    rb = nsm.tile([64, 512], F32, tag="rb", name=f"rbt{s}")
                    nc.gpsimd.partition_broadcast(rb[:], rfs[s][:])
                    rbs[s] = rb
                for ch in range(4):
                    for s in range(2):
                        cs = slice(qb * 512 + ch * 128, qb * 512 + ch * 128 + 128)
                        nc.vector.tensor_mul(
                            ot_sb[64 * s : 64 * s + 64, 1, cs],
                            psq[s][0:64, ch * 128 : ch * 128 + 128],
                            rbs[s][:, ch * 128 : ch * 128 + 128],
                        )
                # Phase 2: hp1-phase matmuls + casts + output DMAs
                dmaq = [nc.scalar, nc.sync, nc.gpsimd, nc.scalar]
                for i, tt in enumerate(tts[:3]):
                    tloc = slice(tt * 128, tt * 128 + 128)
                    for cb, pc in enumerate(accs[tt]):
                        nc.tensor.matmul(
                            pc[:],
                            ot_sb[:, 1, tloc],
                            wp_sb[:, 1, cb * 512 : cb * 512 + 512],
                            start=False, stop=True,
                        )
                    ob = outst.tile([128, 1024], BF16, tag="ob", name="obt")
                    if i % 2 == 0:
                        nc.scalar.copy(ob[:, 0:512], accs[tt][0][:])
                        nc.scalar.copy(ob[:, 512:1024], accs[tt][1][:])
                    else:
                        nc.vector.tensor_copy(out=ob[:, 0:512], in_=accs[tt][0][:])
                        nc.vector.tensor_copy(out=ob[:, 512:1024], in_=accs[tt][1][:])
                    dmaq[i].dma_start(
                        out=out_p[tt * 128 : tt * 128 + 128, :], in_=ob[:])
                # tt15: both phases back-to-back in the remaining gp slot;
                # its output DMA is split across two queues so the final
                # transfer (the kernel's last op) is half as long
                tt = tts[3]
                tloc = slice(tt * 128, tt * 128 + 128)
                ob = outst.tile([128, 1024], BF16, tag="ob", name="obt3")
                for cb in range(2):
                    pc = gpp.tile([128, 512], F32, tag="gp", name=f"pt3{cb}")
                    for hpp in range(2):
                        nc.tensor.matmul(
                            pc[:],
                            ot_sb[:, hpp, tloc],
                            wp_sb[:, hpp, cb * 512 : cb * 512 + 512],
                            start=(hpp == 0), stop=(hpp == 1),
                        )
                    nc.vector.tensor_copy(
                        out=ob[:, cb * 512 : cb * 512 + 512], in_=pc[:])
                    eng = nc.scalar if cb == 0 else nc.sync
                    eng.dma_start(
                        out=out_p[tt * 128 : tt * 128 + 128,
                                  cb * 512 : cb * 512 + 512],
                        in_=ob[:, cb * 512 : cb * 512 + 512])

            # ---- main pipeline ----------------------------------------
            for _ in gen_qk(0):
                pass
            gq = {}
            fillers.append(gen_v(0))
            for qb in range(NQB):
                if qb < NQB - 1:
                    # stage the next t-block of x now: it rides the Scalar
                    # queue ahead of this block's exps and transfers while
                    # attn(qb) runs, without competing with wq/x0 at start
                    nc.scalar.dma_start(
                        out=xt_all[:, qb + 1].rearrange("p c t -> p (c t)"),
                        in_=xT[:, qb + 1].rearrange("p c t -> p (c t)"))
                    gq[qb + 1] = gen_qk(qb + 1)
                    fillers.append(gq[qb + 1])
                    fillers.append(gen_v(qb + 1))
                else:
                    # attn(3) is the longest ACT-bound stretch and has no
                    # QKV left to interleave: feed it all three finished
                    # projection blocks to keep the PE warm.  attn(2)'s
                    # deferred normalize must be flushed first: gen_proj(2)
                    # reads the O^T columns it writes, and a filler chunk
                    # emitted before it would read pre-normalize data.
                    if pending["fn"] is not None:
                        pending["fn"]()
                        pending["fn"] = None
                    for pq_ in range(NQB - 1):
                        fillers.append(gen_proj(pq_))
                emit_attn(qb, tail_hook=tail if qb == NQB - 1 else None)
                # attn(qb+1) S-matmuls read qt/kt of t-block qb+1, so those
                # must be fully emitted first; V is only read by late P@V
                # groups and can keep riding as filler.
                if qb < NQB - 1:
                    drain(gq[qb + 1])

    nc.compile()
    return nc


_NC_CACHE = None


def _get_program():
    global _NC_CACHE
    if _NC_CACHE is None:
        _NC_CACHE = build_program()
    return _NC_CACHE


def make_in_maps(x, Wq, Wk, Wv, Wp):
    import ml_dtypes
    x = np.asarray(x, np.float32)
    Wq = np.asarray(Wq, np.float32)
    Wk = np.asarray(Wk, np.float32)
    Wv = np.asarray(Wv, np.float32)
    Wp = np.asarray(Wp, np.float32)
    tri = np.triu(np.ones((128, 128), np.float32))  # mask[k,q] = (k<=q)
    maskd = np.concatenate([tri, tri], axis=1).astype(ml_dtypes.bfloat16)
    in_maps = []
    for core in range(NCORES):
        b, hg = core // 4, core % 4
        sel = slice(hg * DSEL, (hg + 1) * DSEL)
        # SBUF images: [128, cc, ...] with partition index innermost in
        # the original feature dim (feature c -> (cc, p))
        # [128, cc, T] -> [128, tb, cc, 512] (contiguous per t-block)
        xi = x[b].T.reshape(NCC, 128, T).transpose(1, 0, 2)
        xi = xi.reshape(128, NCC, NQB, 512).transpose(0, 2, 1, 3)
        wqi = Wq[sel, :].T.reshape(NCC, 128, DSEL).transpose(1, 0, 2).reshape(128, NCC * DSEL)
        wki = Wk[sel, :].T.reshape(NCC, 128, DSEL).transpose(1, 0, 2).reshape(128, NCC * DSEL)
        wvi = Wv[sel, :].T.reshape(NCC, 128, DSEL).transpose(1, 0, 2).reshape(128, NCC * DSEL)
        wpi = Wp[:, sel].T.reshape(2, 128, C).transpose(1, 0, 2).reshape(128, 2 * C)
        in_maps.append({
            "xT": np.ascontiguousarray(xi.astype(ml_dtypes.bfloat16)),
            "wqT": np.ascontiguousarray(wqi.astype(ml_dtypes.bfloat16)),
            "wkT": np.ascontiguousarray(wki.astype(ml_dtypes.bfloat16)),
            "wvT": np.ascontiguousarray(wvi.astype(ml_dtypes.bfloat16)),
            "wpT": np.ascontiguousarray(wpi.astype(ml_dtypes.bfloat16)),
            "maskd": maskd,
        })
    return in_maps


def combine_outputs(results, bp):
    parts = [np.asarray(results[i]["out_p"], np.float32) for i in range(NCORES)]
    out = np.stack([
        parts[0] + parts[1] + parts[2] + parts[3],
        parts[4] + parts[5] + parts[6] + parts[7],
    ])
    return (out + np.asarray(bp, np.float32)).astype(np.float32)


def kernel(x, Wq, Wk, Wv, Wp, bp):
    nc = _get_program()
    in_maps = make_in_maps(x, Wq, Wk, Wv, Wp)
    res = bass_utils.run_bass_kernel_spmd(nc, in_maps, core_ids=list(range(NCORES)))
    return combine_outputs(res.results, bp)
